# revision 10
# baseline (speedup 1.0000x reference)
"""Trainium2 Bass kernel for BilinearSeqAttnAction:

    w = weight[actions]              # [B, Y, X]
    Wy = einsum('by,byx->bx', y, w) + bias[actions]
    xWy = einsum('blx,bx->bl', x, Wy)
    alpha = log_softmax(where(x_mask, -inf, xWy), axis=-1)

Sharding (8 NeuronCores, fully data-parallel, no collectives):
  The host packs batches onto cores grouped by action (a batch's slot
  assignment is pure indexing), so each core's 16 batches span only ~4-5
  distinct actions.  Each core loads just those dedup'd weight matrices
  (G slots of 2MB bf16; G = max distinct over cores, same program on all
  cores) and computes Wy for its own batches with masked yT columns:
  slot g's lhsT holds y[b] in column lb only if batch lb uses slot g's
  action, so accumulating all slots over the contraction dim yields each
  batch's own y @ weight[action].  Stage 2 streams the core's 16
  host-transposed x[b] slices ([X, L] so the X-contraction runs on the
  TensorEngine in single-pass bf16) against WyT columns; scores
  accumulate [16, L] f32 in PSUM with batch on partitions, so the
  log_softmax tail is per-partition free-dim work.  The x stream is
  sync-gated behind the last weight DMA so stage 1 gets the full HBM
  bandwidth up front; outputs are unsorted on the host.
"""

import numpy as np

from ml_dtypes import bfloat16 as _np_bf16

N_CORES = 8
B, L, X, Y, A = 128, 1024, 1024, 1024, 32
BC = B // N_CORES  # batches per core
CH = X // 128      # 128-wide chunks of the contraction dim
P = 128

_cached = {}


def _build_program(G):
    from concourse import bacc, tile, mybir
    from concourse.tile import add_dep_helper

    f32 = mybir.dt.float32
    bf16 = mybir.dt.bfloat16
    u8 = mybir.dt.uint8
    AF = mybir.ActivationFunctionType

    nc = bacc.Bacc(
        "TRN2",
        target_bir_lowering=False,
        debug=False,
        enable_asserts=False,
        num_devices=N_CORES,
    )

    xt_d = nc.dram_tensor("xt", [BC, P, CH * L], bf16, kind="ExternalInput").ap()
    ytm_d = nc.dram_tensor("ytm", [P, G * CH * BC], bf16, kind="ExternalInput").ap()
    wt_d = nc.dram_tensor("wt", [G, P, CH * X], bf16, kind="ExternalInput").ap()
    biasg_d = nc.dram_tensor("biasg", [BC, X], f32, kind="ExternalInput").ap()
    mask_d = nc.dram_tensor("masku", [BC, L], u8, kind="ExternalInput").ap()
    eye_d = nc.dram_tensor("eye16", [BC, BC], f32, kind="ExternalInput").ap()
    out_d = nc.dram_tensor("out", [BC, L], f32, kind="ExternalOutput").ap()

    # per-partition SBUF budget is ~192KB; weights take 16KB per slot and
    # half-batch x tiles 8KB per slot
    w_bufs = min(G, 6)
    x_bufs = max(4, (192 - 36 - 16 * w_bufs) // 8)

    with tile.TileContext(nc) as tc:
        with (
            tc.tile_pool(name="ypool", bufs=1) as ypool,
            tc.tile_pool(name="wpool", bufs=w_bufs) as wpool,
            tc.tile_pool(name="tmppool", bufs=2) as tmppool,
            tc.tile_pool(name="xpool", bufs=x_bufs) as xpool,
            tc.tile_pool(name="small", bufs=1) as small,
            tc.tile_pool(name="ps_wy", bufs=1, space="PSUM") as ps_wy_pool,
            tc.tile_pool(name="ps_t", bufs=1, space="PSUM") as ps_t_pool,
            tc.tile_pool(name="ps_s", bufs=2, space="PSUM") as ps_s_pool,
        ):
            # ---- Stage 1: Wy for this core's own 16 batches over its G
            # dedup'd weight slots.  One DMA for all of ytm, one 2MB DMA per
            # slot, all issued first on the SP ring.
            yt_all = ypool.tile([P, G * CH * BC], bf16)
            nc.sync.dma_start(yt_all[:], ytm_d[:])
            w_tiles = []
            w_last_dma = None
            for g in range(G):
                w_t = wpool.tile([P, CH * X], bf16, name="w_t", tag="w_t")
                w_last_dma = nc.sync.dma_start(w_t[:], wt_d[g])
                w_tiles.append(w_t)

            ps_wy = ps_wy_pool.tile([BC, X], f32)
            for g in range(G):
                for k in range(CH):
                    lhs = yt_all[:, (g * CH + k) * BC : (g * CH + k + 1) * BC]
                    for j in range(2):
                        nc.tensor.matmul(
                            ps_wy[:, j * 512 : (j + 1) * 512],
                            lhs,
                            w_tiles[g][:, k * X + j * 512 : k * X + (j + 1) * 512],
                            start=(g == 0 and k == 0),
                            stop=(g == G - 1 and k == CH - 1),
                        )

            bias_sb = small.tile([BC, X], f32)
            nc.scalar.dma_start(bias_sb[:], biasg_d[:])
            wy_sb = small.tile([BC, X], f32)
            nc.vector.tensor_add(wy_sb[:], ps_wy[:], bias_sb[:])

            # ---- Transpose Wy [16, X] -> WyT [X-chunk partitions, 16] so it
            # can be the stationary matmul operand of stage 2 (cast to bf16).
            eye_sb = small.tile([BC, BC], f32)
            nc.scalar.dma_start(eye_sb[:], eye_d[:])
            warm_in = small.tile([BC, 1], f32)
            nc.vector.memset(warm_in[:], 1.0)
            warm_out = small.tile([BC, 1], f32)
            nc.scalar.activation(warm_out[:], warm_in[:], AF.Exp)
            nc.scalar.activation(warm_out[:], warm_in[:], AF.Ln)
            ps_t = ps_t_pool.tile([P, CH * BC], f32)
            for c in range(CH):
                nc.tensor.transpose(
                    ps_t[:, c * BC : (c + 1) * BC],
                    wy_sb[:, c * P : (c + 1) * P],
                    eye_sb[:],
                )
            wyT = small.tile([P, CH * BC], bf16)
            nc.scalar.copy(wyT[:], ps_t[:])

            # ---- Stage 2: scores[b, l] = sum_x xT[b][x, l] * Wy[b, x].
            # lhsT holds ALL 16 Wy columns, so each matmul produces the full
            # 16xL cross-product; only row b is the real batch-b result.
            # Compute engines can't address single partitions (32-aligned
            # base rule), so copy the whole block to SBUF and let a DMA
            # gather row b into the scores tile.
            scores = small.tile([BC, L], f32)
            mask_sb = small.tile([BC, L], u8)
            nc.scalar.dma_start(mask_sb[:], mask_d[:])
            neg_sb = small.tile([BC, L], f32)
            nc.vector.memset(neg_sb[:], float("-inf"))
            HC = CH // 2
            for b in range(BC):
                xhs = []
                for h in range(2):
                    xh = xpool.tile([P, HC * L], bf16, name="xb", tag="xb")
                    x_dma = nc.sync.dma_start(
                        xh[:], xt_d[b, :, h * HC * L : (h + 1) * HC * L]
                    )
                    add_dep_helper(
                        x_dma.ins,
                        w_last_dma.ins,
                        sync=True,
                        reason="x stream yields HBM bandwidth to stage-1 weights",
                    )
                    xhs.append(xh)
                ps_s = ps_s_pool.tile([BC, L], f32, name="ps_s", tag="ps_s")
                for c in range(CH):
                    xh = xhs[c // HC]
                    cl = c % HC
                    for j in range(2):
                        nc.tensor.matmul(
                            ps_s[:, j * 512 : (j + 1) * 512],
                            wyT[:, c * BC : (c + 1) * BC],
                            xh[:, cl * L + j * 512 : cl * L + (j + 1) * 512],
                            start=(c == 0),
                            stop=(c == CH - 1),
                        )
                tmp = tmppool.tile([BC, L], f32, name="tmp", tag="tmp")
                nc.scalar.copy(tmp[:], ps_s[:])
                nc.vector.copy_predicated(tmp[:], mask_sb[:], neg_sb[:])
                nc.scalar.dma_start(scores[b : b + 1, :], tmp[b : b + 1, :])

            # ---- log_softmax, batch on partitions throughout.
            negm = small.tile([BC, 1], f32)
            nc.vector.reduce_max(
                negm[:], scores[:], axis=mybir.AxisListType.X, negate=True
            )
            scratch = small.tile([BC, L], f32)
            ssum = small.tile([BC, 1], f32)
            nc.scalar.activation(
                scratch[:], scores[:], AF.Exp, bias=negm[:], scale=1.0, accum_out=ssum[:]
            )
            lg = small.tile([BC, 1], f32)
            nc.scalar.activation(lg[:], ssum[:], AF.Ln)
            shift = small.tile([BC, 1], f32)
            nc.vector.tensor_sub(shift[:], negm[:], lg[:])
            nc.vector.tensor_scalar_add(scratch[:], scores[:], shift[:])
            nc.scalar.dma_start(out_d[:], scratch[:])

    nc.compile()
    return nc


def _get_program(G):
    key = ("nc", G)
    if key not in _cached:
        _cached[key] = _build_program(G)
    return _cached[key]


def _pack_batches(actions):
    """Assign batches to cores grouped by action so each core sees few
    distinct actions.  Greedy largest-group-first bin packing with group
    splitting; returns (order, slots) where order is the batch permutation
    (16 per core) and slots[i] is core i's list of (action, local batch
    indices) weight slots."""
    groups = {}
    for b, a in enumerate(actions.tolist()):
        groups.setdefault(a, []).append(b)
    order_groups = sorted(groups.items(), key=lambda kv: -len(kv[1]))
    free = [BC] * N_CORES
    slots = [[] for _ in range(N_CORES)]
    for a, bs in order_groups:
        rest = bs
        while rest:
            i = max(range(N_CORES), key=lambda c: free[c])
            take = min(free[i], len(rest))
            slots[i].append((a, rest[:take]))
            free[i] -= take
            rest = rest[take:]
    order = []
    for i in range(N_CORES):
        for a, bs in slots[i]:
            order.extend(bs)
    return np.array(order), slots


def kernel(**inputs) -> np.ndarray:
    x = np.asarray(inputs["x"], dtype=np.float32)
    y = np.asarray(inputs["y"], dtype=np.float32)
    x_mask = np.asarray(inputs["x_mask"])
    actions = np.asarray(inputs["actions"]).astype(np.int64)
    weight = np.asarray(inputs["weight"], dtype=np.float32)
    bias = np.asarray(inputs["bias"], dtype=np.float32)

    order, slots = _pack_batches(actions)
    G = max(len(s) for s in slots)
    nc = _get_program(G)
    eye = np.eye(BC, dtype=np.float32)
    wbf = weight.reshape(A, CH, P, X).transpose(0, 2, 1, 3).astype(_np_bf16)

    in_maps = []
    for i in range(N_CORES):
        sel = order[BC * i : BC * (i + 1)]
        # Single-stride device layouts: [..., p, k, inner] so each DMA is a
        # plain 2D transfer with 16KB contiguous partition lines.
        xt = np.ascontiguousarray(
            x[sel].reshape(BC, L, CH, P).transpose(0, 3, 2, 1)
        ).astype(_np_bf16).reshape(BC, P, CH * L)
        ytm = np.zeros((G, Y, BC), dtype=np.float32)
        wt = np.zeros((G, P, CH * X), dtype=_np_bf16)
        base = 0
        for g, (a, bs) in enumerate(slots[i]):
            lbs = list(range(base, base + len(bs)))
            base += len(bs)
            ytm[g][:, lbs] = y[bs].T
            wt[g] = wbf[a].reshape(P, CH * X)
        for g in range(len(slots[i]), G):  # pad slots: zero mask, any weight
            wt[g] = wt[0]
        ytm = np.ascontiguousarray(
            ytm.reshape(G, CH, P, BC).transpose(2, 0, 1, 3)
        ).reshape(P, G * CH * BC)
        in_maps.append(
            {
                "xt": xt,
                "ytm": ytm.astype(_np_bf16),
                "wt": wt,
                "biasg": np.ascontiguousarray(bias[actions[sel]]),
                "masku": x_mask[sel].astype(np.uint8),
                "eye16": eye,
            }
        )

    from concourse import bass_utils

    res = bass_utils.run_bass_kernel_spmd(
        nc, in_maps, core_ids=list(range(N_CORES))
    )
    _cached["last_results"] = res
    out_sorted = np.concatenate(
        [res.results[i]["out"] for i in range(N_CORES)], axis=0
    )
    out = np.empty_like(out_sorted)
    out[order] = out_sorted
    return out


# revision 12
# speedup vs baseline: 1.1232x; 1.1232x over previous
"""Trainium2 Bass kernel for BilinearSeqAttnAction:

    w = weight[actions]              # [B, Y, X]
    Wy = einsum('by,byx->bx', y, w) + bias[actions]
    xWy = einsum('blx,bx->bl', x, Wy)
    alpha = log_softmax(where(x_mask, -inf, xWy), axis=-1)

Sharding (8 NeuronCores, fully data-parallel, no collectives):
  The host packs batches onto cores grouped by action (a batch's slot
  assignment is pure indexing), so each core's 16 batches span only ~4-5
  distinct actions.  Each core loads just those dedup'd weight matrices
  (G slots of 2MB bf16; G = max distinct over cores, same program on all
  cores) and computes Wy for its own batches with masked yT columns:
  slot g's lhsT holds y[b] in column lb only if batch lb uses slot g's
  action, so accumulating all slots over the contraction dim yields each
  batch's own y @ weight[action].  Stage 2 streams the core's 16
  host-transposed x[b] slices ([X, L] so the X-contraction runs on the
  TensorEngine in single-pass bf16) against WyT columns; scores
  accumulate [16, L] f32 in PSUM with batch on partitions, so the
  log_softmax tail is per-partition free-dim work.  Outputs are unsorted
  on the host.
"""

import numpy as np

from ml_dtypes import bfloat16 as _np_bf16

N_CORES = 8
B, L, X, Y, A = 128, 1024, 1024, 1024, 32
BC = B // N_CORES  # batches per core
CH = X // 128      # 128-wide chunks of the contraction dim
P = 128

_cached = {}


def _build_program(G):
    from concourse import bacc, tile, mybir

    f32 = mybir.dt.float32
    bf16 = mybir.dt.bfloat16
    u8 = mybir.dt.uint8
    AF = mybir.ActivationFunctionType

    nc = bacc.Bacc(
        "TRN2",
        target_bir_lowering=False,
        debug=False,
        enable_asserts=False,
        num_devices=N_CORES,
    )

    xt_d = nc.dram_tensor("xt", [BC * 2, P, CH * L // 2], bf16, kind="ExternalInput").ap()
    ytm_d = nc.dram_tensor("ytm", [P, G * CH * BC], bf16, kind="ExternalInput").ap()
    wt_d = nc.dram_tensor("wt", [G, P, CH * X], bf16, kind="ExternalInput").ap()
    biasg_d = nc.dram_tensor("biasg", [BC, X], f32, kind="ExternalInput").ap()
    mask_d = nc.dram_tensor("masku", [BC, L], u8, kind="ExternalInput").ap()
    eye_d = nc.dram_tensor("eye16", [BC, BC], f32, kind="ExternalInput").ap()
    out_d = nc.dram_tensor("out", [BC, L], f32, kind="ExternalOutput").ap()

    # per-partition SBUF budget is ~192KB; weights take 16KB per slot and
    # half-batch x tiles 8KB per slot
    w_bufs = min(G, 6)
    x_bufs = max(4, (192 - 36 - 16 * w_bufs) // 8)

    with tile.TileContext(nc) as tc:
        with (
            tc.tile_pool(name="ypool", bufs=1) as ypool,
            tc.tile_pool(name="wpool", bufs=w_bufs) as wpool,
            tc.tile_pool(name="tmppool", bufs=2) as tmppool,
            tc.tile_pool(name="xpool", bufs=x_bufs) as xpool,
            tc.tile_pool(name="small", bufs=1) as small,
            tc.tile_pool(name="ps_wy", bufs=1, space="PSUM") as ps_wy_pool,
            tc.tile_pool(name="ps_t", bufs=1, space="PSUM") as ps_t_pool,
            tc.tile_pool(name="ps_s", bufs=2, space="PSUM") as ps_s_pool,
        ):
            # ---- Stage 1: Wy for this core's own 16 batches over its G
            # dedup'd weight slots.  One DMA for all of ytm, one 2MB DMA per
            # slot, all issued first on the SP ring.
            yt_all = ypool.tile([P, G * CH * BC], bf16)
            nc.sync.dma_start(yt_all[:], ytm_d[:])
            w_tiles = []
            for g in range(G):
                w_t = wpool.tile([P, CH * X], bf16, name="w_t", tag="w_t")
                nc.sync.dma_start(w_t[:], wt_d[g])
                w_tiles.append(w_t)

            ps_wy = ps_wy_pool.tile([BC, X], f32)
            for g in range(G):
                for k in range(CH):
                    lhs = yt_all[:, (g * CH + k) * BC : (g * CH + k + 1) * BC]
                    for j in range(2):
                        nc.tensor.matmul(
                            ps_wy[:, j * 512 : (j + 1) * 512],
                            lhs,
                            w_tiles[g][:, k * X + j * 512 : k * X + (j + 1) * 512],
                            start=(g == 0 and k == 0),
                            stop=(g == G - 1 and k == CH - 1),
                        )

            bias_sb = small.tile([BC, X], f32)
            nc.scalar.dma_start(bias_sb[:], biasg_d[:])
            wy_sb = small.tile([BC, X], f32)
            nc.vector.tensor_add(wy_sb[:], ps_wy[:], bias_sb[:])

            # ---- Transpose Wy [16, X] -> WyT [X-chunk partitions, 16] so it
            # can be the stationary matmul operand of stage 2 (cast to bf16).
            eye_sb = small.tile([BC, BC], f32)
            nc.scalar.dma_start(eye_sb[:], eye_d[:])
            warm_in = small.tile([BC, 1], f32)
            nc.vector.memset(warm_in[:], 1.0)
            warm_out = small.tile([BC, 1], f32)
            nc.scalar.activation(warm_out[:], warm_in[:], AF.Exp)
            nc.scalar.activation(warm_out[:], warm_in[:], AF.Ln)
            ps_t = ps_t_pool.tile([P, CH * BC], f32)
            for c in range(CH):
                nc.tensor.transpose(
                    ps_t[:, c * BC : (c + 1) * BC],
                    wy_sb[:, c * P : (c + 1) * P],
                    eye_sb[:],
                )
            wyT = small.tile([P, CH * BC], bf16)
            nc.scalar.copy(wyT[:], ps_t[:])

            # ---- Stage 2: scores[b, l] = sum_x xT[b][x, l] * Wy[b, x].
            # lhsT holds ALL 16 Wy columns, so each matmul produces the full
            # 16xL cross-product; only row b is the real batch-b result.
            # Compute engines can't address single partitions (32-aligned
            # base rule), so copy the whole block to SBUF and let a DMA
            # gather row b into the scores tile.
            scores = small.tile([BC, L], f32)
            mask_sb = small.tile([BC, L], u8)
            nc.scalar.dma_start(mask_sb[:], mask_d[:])
            neg_sb = small.tile([BC, L], f32)
            nc.vector.memset(neg_sb[:], float("-inf"))
            HC = CH // 2
            for b in range(BC):
                xhs = []
                for h in range(2):
                    xh = xpool.tile([P, HC * L], bf16, name="xb", tag="xb")
                    nc.sync.dma_start(xh[:], xt_d[2 * b + h])
                    xhs.append(xh)
                ps_s = ps_s_pool.tile([BC, L], f32, name="ps_s", tag="ps_s")
                for c in range(CH):
                    xh = xhs[c // HC]
                    cl = c % HC
                    for j in range(2):
                        nc.tensor.matmul(
                            ps_s[:, j * 512 : (j + 1) * 512],
                            wyT[:, c * BC : (c + 1) * BC],
                            xh[:, cl * L + j * 512 : cl * L + (j + 1) * 512],
                            start=(c == 0),
                            stop=(c == CH - 1),
                        )
                tmp = tmppool.tile([BC, L], f32, name="tmp", tag="tmp")
                nc.scalar.copy(tmp[:], ps_s[:])
                nc.vector.copy_predicated(tmp[:], mask_sb[:], neg_sb[:])
                nc.scalar.dma_start(scores[b : b + 1, :], tmp[b : b + 1, :])

            # ---- log_softmax, batch on partitions throughout.
            negm = small.tile([BC, 1], f32)
            nc.vector.reduce_max(
                negm[:], scores[:], axis=mybir.AxisListType.X, negate=True
            )
            scratch = small.tile([BC, L], f32)
            ssum = small.tile([BC, 1], f32)
            nc.scalar.activation(
                scratch[:], scores[:], AF.Exp, bias=negm[:], scale=1.0, accum_out=ssum[:]
            )
            lg = small.tile([BC, 1], f32)
            nc.scalar.activation(lg[:], ssum[:], AF.Ln)
            shift = small.tile([BC, 1], f32)
            nc.vector.tensor_sub(shift[:], negm[:], lg[:])
            nc.vector.tensor_scalar_add(scratch[:], scores[:], shift[:])
            nc.scalar.dma_start(out_d[:], scratch[:])

    nc.compile()
    return nc


def _get_program(G):
    key = ("nc", G)
    if key not in _cached:
        _cached[key] = _build_program(G)
    return _cached[key]


def _pack_batches(actions):
    """Assign batches to cores grouped by action so each core sees few
    distinct actions.  Greedy largest-group-first bin packing with group
    splitting; returns (order, slots) where order is the batch permutation
    (16 per core) and slots[i] is core i's list of (action, local batch
    indices) weight slots."""
    groups = {}
    for b, a in enumerate(actions.tolist()):
        groups.setdefault(a, []).append(b)
    order_groups = sorted(groups.items(), key=lambda kv: -len(kv[1]))
    free = [BC] * N_CORES
    slots = [[] for _ in range(N_CORES)]
    for a, bs in order_groups:
        rest = bs
        while rest:
            i = max(range(N_CORES), key=lambda c: free[c])
            take = min(free[i], len(rest))
            slots[i].append((a, rest[:take]))
            free[i] -= take
            rest = rest[take:]
    order = []
    for i in range(N_CORES):
        for a, bs in slots[i]:
            order.extend(bs)
    return np.array(order), slots


def kernel(**inputs) -> np.ndarray:
    x = np.asarray(inputs["x"], dtype=np.float32)
    y = np.asarray(inputs["y"], dtype=np.float32)
    x_mask = np.asarray(inputs["x_mask"])
    actions = np.asarray(inputs["actions"]).astype(np.int64)
    weight = np.asarray(inputs["weight"], dtype=np.float32)
    bias = np.asarray(inputs["bias"], dtype=np.float32)

    order, slots = _pack_batches(actions)
    G = max(len(s) for s in slots)
    nc = _get_program(G)
    eye = np.eye(BC, dtype=np.float32)
    wbf = weight.reshape(A, CH, P, X).transpose(0, 2, 1, 3).astype(_np_bf16)

    in_maps = []
    for i in range(N_CORES):
        sel = order[BC * i : BC * (i + 1)]
        # Single-stride device layouts: [..., p, k, inner] so each DMA is a
        # plain 2D transfer with 16KB contiguous partition lines.
        xt = np.ascontiguousarray(
            x[sel].reshape(BC, L, 2, CH // 2, P).transpose(0, 2, 4, 3, 1)
        ).astype(_np_bf16).reshape(BC * 2, P, CH * L // 2)
        ytm = np.zeros((G, Y, BC), dtype=np.float32)
        wt = np.zeros((G, P, CH * X), dtype=_np_bf16)
        base = 0
        for g, (a, bs) in enumerate(slots[i]):
            lbs = list(range(base, base + len(bs)))
            base += len(bs)
            ytm[g][:, lbs] = y[bs].T
            wt[g] = wbf[a].reshape(P, CH * X)
        for g in range(len(slots[i]), G):  # pad slots: zero mask, any weight
            wt[g] = wt[0]
        ytm = np.ascontiguousarray(
            ytm.reshape(G, CH, P, BC).transpose(2, 0, 1, 3)
        ).reshape(P, G * CH * BC)
        in_maps.append(
            {
                "xt": xt,
                "ytm": ytm.astype(_np_bf16),
                "wt": wt,
                "biasg": np.ascontiguousarray(bias[actions[sel]]),
                "masku": x_mask[sel].astype(np.uint8),
                "eye16": eye,
            }
        )

    from concourse import bass_utils

    res = None
    last_err = None
    for _attempt in range(3):
        try:
            res = bass_utils.run_bass_kernel_spmd(
                nc, in_maps, core_ids=list(range(N_CORES))
            )
            break
        except Exception as e:  # transient device/runtime hiccups
            last_err = e
    if res is None:
        raise last_err
    _cached["last_results"] = res
    out_sorted = np.concatenate(
        [res.results[i]["out"] for i in range(N_CORES)], axis=0
    )
    out = np.empty_like(out_sorted)
    out[order] = out_sorted
    return out
